# revision 25
# baseline (speedup 1.0000x reference)
"""Trainium2 Bass kernel for the kNN pairwise-ranking loss.

Math: with y = (knn_tgts == tgts), the masked pairwise BCE-with-logits loss
over differing-label pairs (j > i) collapses to

    loss = sum_b sum_{n in neg_b} sum_{p in pos_b} softplus(s_n - s_p) / cnt
    cnt  = sum_b |pos_b| * |neg_b|

because for a (pos, neg) pair the per-pair term is softplus(s_neg - s_pos)
regardless of orientation, and b2 cancels in score differences.

Host side: per batch row, permute keys so positives come first, then
negatives, then masked-out entries.  Additive pad vectors (+PAD on
non-positives, -PAD on non-negatives) push padded scores far out so their
softplus contribution underflows to exactly ln(1) = 0; the device then just
sums a dense [pos-chunks x neg-width] softplus block with no masking.

Device (SPMD over 8 cores, 4 batch rows each):
  phase A (per row): h = relu(W1 @ keys^T + b1) via PE (bf16 in, f32 psum),
                     s_row [1,K] via two small PE matmuls off the bf16 h.
  phase B (per row): softplus(s_n - s_p) = ln(1 + e^{s_n} * e^{-s_p}).
                     One fused ACT exp produces both e^{-(s_pos+pad)} [1,Jmax]
                     and e^{s_neg+pad} [1,nw]; GPSIMD partition-broadcasts the
                     row factor, a DRAM round-trip reshapes the column factor
                     to [128,npch]; DVE per-partition multiplies form the
                     outer products; one Ln(x+1) pass per row with accum_out
                     yields the per-partition sums.  Exp and Ln are forced
                     into one ACT table set (see _patch_act_tables) so only a
                     single ACT_TABLE_LOAD is issued.
Host gathers [128, BPC] partial sums, reduces, divides by cnt.
"""

import numpy as np

B, K, D, H = 32, 1024, 1024, 100
N_CORES = 8
BPC = B // N_CORES  # batch rows per core
PAD = 60.0
USE_BF16 = True

_cache = {}
_act_patched = False


def _patch_act_tables():
    """Make Exp/Ln resolve to the single combined ACT table set."""
    global _act_patched
    if _act_patched:
        return
    import concourse.bacc as bacc
    import concourse.hw_specs as hw_specs
    import concourse.mybir as mybir

    orig = hw_specs.get_activation_tables
    combined = "natural_log_exp_and_others"

    def patched(arch):
        tabs = orig(arch)
        out = {}
        for name, funcs in tabs.items():
            f = set(funcs)
            if name != combined and combined in tabs:
                f.discard(mybir.ActivationFunctionType.Exp)
                f.discard(mybir.ActivationFunctionType.Ln)
            out[name] = f
        return out

    hw_specs.get_activation_tables = patched
    bacc.get_activation_tables = patched
    _act_patched = True


def _build_program(Jmax, nst, use_bf16):
    import concourse.bacc as bacc
    import concourse.mybir as mybir
    import concourse.tile as tile

    _patch_act_tables()

    f32 = mybir.dt.float32
    kdt = mybir.dt.bfloat16 if use_bf16 else f32
    npch = Jmax // 128  # positive-side partition chunks
    nw = K - nst  # negative-side free width

    nc = bacc.Bacc(
        "TRN2",
        target_bir_lowering=False,
        debug=False,
        enable_asserts=False,
        num_devices=N_CORES,
    )

    keys_d = nc.dram_tensor("keys_t", [BPC, D, K], kdt, kind="ExternalInput").ap()
    w1t_d = nc.dram_tensor("w1t", [D, H], kdt, kind="ExternalInput").ap()
    w2_d = nc.dram_tensor("w2c", [H, 1], kdt, kind="ExternalInput").ap()
    b1_d = nc.dram_tensor("b1c", [H, 1], f32, kind="ExternalInput").ap()
    ppr_d = nc.dram_tensor("ppr", [BPC, Jmax], f32, kind="ExternalInput").ap()
    negrow_d = nc.dram_tensor("negrow", [BPC, nw], f32, kind="ExternalInput").ap()
    out_d = nc.dram_tensor(
        "acc_out", [128, BPC], f32, kind="ExternalOutput"
    ).ap()

    with tile.TileContext(nc) as tc:
        with (
            tc.tile_pool(name="const", bufs=1) as cpool,
            tc.tile_pool(name="keys", bufs=28) as kpool,
            tc.tile_pool(name="h", bufs=4) as hpool,
            tc.tile_pool(name="svec", bufs=3) as spool,
            tc.tile_pool(name="big", bufs=3) as bpool,
            tc.tile_pool(name="dscr", bufs=2, space="DRAM") as dpool,
            tc.tile_pool(name="hp", bufs=3, space="PSUM") as hp_pool,
        ):
            # ---- constants (scalar+gpsimd queues: sync queue is for keys) ----
            w1t_sb = cpool.tile([128, 8 * H], kdt, tag="w1t")
            for dc in range(8):
                eng = nc.scalar if dc % 2 == 0 else nc.gpsimd
                eng.dma_start(
                    w1t_sb[:, dc * H : (dc + 1) * H],
                    w1t_d[dc * 128 : (dc + 1) * 128, :],
                )
            w2_sb = cpool.tile([H, 1], kdt, tag="w2")
            nc.scalar.dma_start(w2_sb[:], w2_d[:])
            b1_sb = cpool.tile([H, 1], f32, tag="b1")
            nc.scalar.dma_start(b1_sb[:], b1_d[:])
            acc_sb = cpool.tile([128, BPC], f32, tag="acc")

            for b in range(BPC):
                # ---- phase A: MLP scores ----
                hp = hp_pool.tile([H, 1024], f32, tag="hp")
                for dc in range(8):
                    kt = kpool.tile([128, K], kdt, tag="keys")
                    nc.sync.dma_start(kt[:], keys_d[b, dc * 128 : (dc + 1) * 128, :])
                    w_sl = w1t_sb[:, dc * H : (dc + 1) * H]
                    nc.tensor.matmul(
                        hp[:, 0:512], lhsT=w_sl, rhs=kt[:, 0:512],
                        start=(dc == 0), stop=(dc == 7),
                    )
                    nc.tensor.matmul(
                        hp[:, 512:1024], lhsT=w_sl, rhs=kt[:, 512:1024],
                        start=(dc == 0), stop=(dc == 7),
                    )
                # relu(h + b1): PSUM -> SBUF on DVE (cast to kdt for matmul-2)
                h0 = hpool.tile([H, 512], kdt, tag="h")
                h1 = hpool.tile([H, 512], kdt, tag="h")
                nc.vector.tensor_scalar(
                    h0[:], hp[:, 0:512], b1_sb[:], 0.0,
                    op0=mybir.AluOpType.add, op1=mybir.AluOpType.max,
                )
                nc.vector.tensor_scalar(
                    h1[:], hp[:, 512:1024], b1_sb[:], 0.0,
                    op0=mybir.AluOpType.add, op1=mybir.AluOpType.max,
                )
                # s_row over the full row (shares PSUM slots with hp)
                sr_ps = hp_pool.tile([1, 1024], f32, tag="hp")
                nc.tensor.matmul(
                    sr_ps[0:1, 0:512], lhsT=w2_sb[:], rhs=h0[:],
                    start=True, stop=True,
                )
                nc.tensor.matmul(
                    sr_ps[0:1, 512:1024], lhsT=w2_sb[:], rhs=h1[:],
                    start=True, stop=True,
                )
                # padded score rows -> one fused tile: [-(s_pos+pad), s_neg+pad]
                ppr_sb = spool.tile([1, Jmax], f32, tag="ppr")
                nc.gpsimd.dma_start(ppr_sb[:], ppr_d[b : b + 1, :])
                ngr_sb = spool.tile([1, nw], f32, tag="ngr")
                nc.gpsimd.dma_start(ngr_sb[:], negrow_d[b : b + 1, :])
                exin_sb = spool.tile([1, Jmax + nw], f32, tag="exin")
                nc.vector.scalar_tensor_tensor(
                    exin_sb[0:1, 0:Jmax], sr_ps[0:1, 0:Jmax], -1.0, ppr_sb[:],
                    op0=mybir.AluOpType.mult, op1=mybir.AluOpType.subtract,
                )
                nc.vector.tensor_add(
                    exin_sb[0:1, Jmax : Jmax + nw], sr_ps[0:1, nst:K], ngr_sb[:]
                )
                # e^{-(s_pos+pad)} | e^{s_neg+pad} in one bf16 row
                eall_sb = spool.tile([1, Jmax + nw], kdt, tag="eall")
                nc.scalar.activation(
                    eall_sb[:], exin_sb[:],
                    mybir.ActivationFunctionType.Exp, scale=1.0,
                )
                # broadcast e^{s_neg} across partitions (GPSIMD, off PE/ACT)
                ebc_sb = bpool.tile([128, nw], kdt, tag="ebc")
                nc.gpsimd.partition_broadcast(
                    ebc_sb[:], eall_sb[0:1, Jmax : Jmax + nw]
                )
                # e^{-(s_pos+pad)} row -> [128, npch] via DRAM round-trip on
                # the scalar queue (never waits behind the keys stream)
                scr = dpool.tile([1, Jmax], kdt, tag="scr")
                nc.scalar.dma_start(scr[:], eall_sb[0:1, 0:Jmax])
                ecc_sb = spool.tile([128, npch], kdt, tag="ecc")
                nc.scalar.dma_start(
                    ecc_sb[:], scr[0:1, :].rearrange("a (c p) -> (a p) c", p=128)
                )
                ecf_sb = spool.tile([128, npch], f32, tag="ecf")
                nc.vector.tensor_copy(ecf_sb[:], ecc_sb[:])
                # outer products on DVE (bf16), one Ln(x+1)+accum per row
                tall_sb = bpool.tile([128, npch * nw], kdt, tag="tall")
                for c in range(npch):
                    nc.vector.tensor_scalar_mul(
                        tall_sb[:, c * nw : (c + 1) * nw], ebc_sb[:],
                        ecf_sb[:, c : c + 1],
                    )
                lout_sb = bpool.tile([128, npch * nw], f32, tag="lout")
                nc.scalar.activation(
                    lout_sb[:], tall_sb[:],
                    mybir.ActivationFunctionType.Ln,
                    bias=1.0, scale=1.0,
                    accum_out=acc_sb[:, b : b + 1],
                )

            nc.sync.dma_start(out_d[:], acc_sb[:])

    nc.compile()
    return nc


def kernel(keys, tgts, knn_tgts, mask, W1, b1, W2, b2, _profile=False):
    import ml_dtypes

    from concourse.bass_utils import run_bass_kernel_spmd

    keys = np.asarray(keys, dtype=np.float32)
    tgts = np.asarray(tgts)
    knn_tgts = np.asarray(knn_tgts)
    mask = np.asarray(mask).astype(bool)
    W1 = np.asarray(W1, dtype=np.float32)
    b1 = np.asarray(b1, dtype=np.float32)
    W2 = np.asarray(W2, dtype=np.float32)

    # ---- host-side label/permutation prep ----
    y = knn_tgts == tgts[:, None]
    pos = y & mask
    neg = (~y) & mask
    P = pos.sum(axis=1)
    N_ = neg.sum(axis=1)
    cnt = float((P.astype(np.int64) * N_.astype(np.int64)).sum())

    # stable order: positives, negatives, masked-out
    rank = np.where(pos, 0, np.where(neg, 1, 2)).astype(np.int8)
    order = np.argsort(rank, axis=1, kind="stable")  # [B, K]

    Pmax = int(P.max())
    Pmin = int(P.min())
    assert Pmax <= 512, f"positive count {Pmax} > 512 unsupported"
    Jmax = min(512, ((Pmax + 127) // 128) * 128)
    npch = Jmax // 128
    nst = min(Pmin, 512)  # negative free region start (s_row slice origin)
    nw = K - nst

    # permuted, transposed keys: [B, D, K]
    keys_perm = np.take_along_axis(keys, order[:, :, None], axis=1)  # [B, K, D]
    keys_t = np.ascontiguousarray(keys_perm.transpose(0, 2, 1))
    kdt = ml_dtypes.bfloat16 if USE_BF16 else np.float32
    keys_t = keys_t.astype(kdt)

    # pads in permuted coordinates
    kidx = np.arange(K)[None, :]
    pospad = np.where(kidx < P[:, None], 0.0, PAD).astype(np.float32)  # [B, K]
    negpad = np.where(
        (kidx >= P[:, None]) & (kidx < (P + N_)[:, None]), 0.0, -PAD
    ).astype(np.float32)
    ppr = np.ascontiguousarray(pospad[:, :Jmax])  # [B, Jmax]
    negrow = np.ascontiguousarray(negpad[:, nst:])  # [B, nw]

    w1t = np.ascontiguousarray(W1.T).astype(kdt)  # [D, H]
    w2c = np.ascontiguousarray(W2.reshape(1, H).T).astype(kdt)  # [H, 1]
    b1c = np.ascontiguousarray(b1.reshape(H, 1))

    key = (Jmax, nst, USE_BF16)
    if key not in _cache:
        _cache[key] = _build_program(Jmax, nst, USE_BF16)
    nc = _cache[key]

    in_maps = []
    for c in range(N_CORES):
        sl = slice(c * BPC, (c + 1) * BPC)
        in_maps.append(
            {
                "keys_t": keys_t[sl],
                "w1t": w1t,
                "w2c": w2c,
                "b1c": b1c,
                "ppr": ppr[sl],
                "negrow": negrow[sl],
            }
        )

    res = run_bass_kernel_spmd(
        nc, in_maps, list(range(N_CORES)), trace=bool(_profile)
    )
    total = 0.0
    for r in res.results:
        total += float(r["acc_out"].astype(np.float64).sum())
    if _profile:
        print(f"HW exec time: {res.exec_time_ns} ns")
        globals()["_last_results"] = res
    loss = np.float64(total) / np.float64(cnt)
    return np.array(loss, dtype=np.float32)


# revision 28
# speedup vs baseline: 1.3402x; 1.3402x over previous
"""Trainium2 Bass kernel for the kNN pairwise-ranking loss.

Math: with y = (knn_tgts == tgts), the masked pairwise BCE-with-logits loss
over differing-label pairs (j > i) collapses to

    loss = sum_b sum_{n in neg_b} sum_{p in pos_b} softplus(s_n - s_p) / cnt
    cnt  = sum_b |pos_b| * |neg_b|

because for a (pos, neg) pair the per-pair term is softplus(s_neg - s_pos)
regardless of orientation, and b2 cancels in score differences.

Host side: per batch row, permute keys so positives come first, then
negatives, then masked-out entries.  Additive pad vectors (+PAD on
non-positives, -PAD on non-negatives) push padded scores far out so their
softplus contribution underflows to exactly ln(1) = 0; the device then just
sums a dense [pos-chunks x neg-width] softplus block with no masking.

Device (SPMD over 8 cores, 4 batch rows each):
  phase A (per row): h = relu(W1 @ keys^T + b1) via PE (bf16 in, f32 psum),
                     s_row [1,K] via two small PE matmuls off the bf16 h.
  phase B (per row): softplus(s_n - s_p) = ln(1 + e^{s_n} * e^{-s_p}).
                     One fused ACT exp produces both e^{-(s_pos+pad)} [1,Jmax]
                     and e^{s_neg+pad} [1,nw]; GPSIMD partition-broadcasts the
                     row factor, a DRAM round-trip reshapes the column factor
                     to [128,npch]; DVE per-partition multiplies form the
                     outer products; one Ln(x+1) pass per row with accum_out
                     yields the per-partition sums.  Exp and Ln are forced
                     into one ACT table set (see _patch_act_tables) so only a
                     single ACT_TABLE_LOAD is issued.
Host gathers [128, BPC] partial sums, reduces, divides by cnt.
"""

import numpy as np

B, K, D, H = 32, 1024, 1024, 100
N_CORES = 8
BPC = B // N_CORES  # batch rows per core
PAD = 60.0
USE_BF16 = True

_cache = {}
_act_patched = False


def _patch_act_tables():
    """Make Exp/Ln resolve to the single combined ACT table set."""
    global _act_patched
    if _act_patched:
        return
    import concourse.bacc as bacc
    import concourse.hw_specs as hw_specs
    import concourse.mybir as mybir

    orig = hw_specs.get_activation_tables
    combined = "natural_log_exp_and_others"

    def patched(arch):
        tabs = orig(arch)
        out = {}
        for name, funcs in tabs.items():
            f = set(funcs)
            if name != combined and combined in tabs:
                f.discard(mybir.ActivationFunctionType.Exp)
                f.discard(mybir.ActivationFunctionType.Ln)
            out[name] = f
        return out

    hw_specs.get_activation_tables = patched
    bacc.get_activation_tables = patched
    _act_patched = True


def _build_program(Jmax, nst, use_bf16):
    import concourse.bacc as bacc
    import concourse.mybir as mybir
    import concourse.tile as tile

    _patch_act_tables()

    f32 = mybir.dt.float32
    kdt = mybir.dt.bfloat16 if use_bf16 else f32
    npch = Jmax // 128  # positive-side partition chunks
    nw = K - nst  # negative-side free width

    nc = bacc.Bacc(
        "TRN2",
        target_bir_lowering=False,
        debug=False,
        enable_asserts=False,
        num_devices=N_CORES,
    )

    keys_d = nc.dram_tensor("keys_t", [BPC, D, K], kdt, kind="ExternalInput").ap()
    w1t_d = nc.dram_tensor("w1t", [D, H], kdt, kind="ExternalInput").ap()
    w2_d = nc.dram_tensor("w2c", [H, 1], kdt, kind="ExternalInput").ap()
    b1_d = nc.dram_tensor("b1c", [H, 1], f32, kind="ExternalInput").ap()
    ppr_d = nc.dram_tensor("ppr", [BPC, Jmax], f32, kind="ExternalInput").ap()
    negrow_d = nc.dram_tensor("negrow", [BPC, nw], f32, kind="ExternalInput").ap()
    out_d = nc.dram_tensor(
        "acc_out", [128, BPC * npch], f32, kind="ExternalOutput"
    ).ap()

    with tile.TileContext(nc) as tc:
        with (
            tc.tile_pool(name="const", bufs=1) as cpool,
            tc.tile_pool(name="keys", bufs=28) as kpool,
            tc.tile_pool(name="h", bufs=4) as hpool,
            tc.tile_pool(name="svec", bufs=3) as spool,
            tc.tile_pool(name="hp", bufs=2, space="PSUM") as hp_pool,
            tc.tile_pool(name="tp", bufs=2, space="PSUM") as tp_pool,
        ):
            # ---- constants (scalar+gpsimd queues: sync queue is for keys) ----
            w1t_sb = cpool.tile([128, 8 * H], kdt, tag="w1t")
            for dc in range(8):
                eng = nc.scalar if dc % 2 == 0 else nc.gpsimd
                eng.dma_start(
                    w1t_sb[:, dc * H : (dc + 1) * H],
                    w1t_d[dc * 128 : (dc + 1) * 128, :],
                )
            w2_sb = cpool.tile([H, 1], kdt, tag="w2")
            nc.scalar.dma_start(w2_sb[:], w2_d[:])
            b1_sb = cpool.tile([H, 1], f32, tag="b1")
            nc.scalar.dma_start(b1_sb[:], b1_d[:])
            acc_sb = cpool.tile([128, BPC * npch], f32, tag="acc")

            for b in range(BPC):
                # ---- phase A: MLP scores ----
                hp = hp_pool.tile([H, 1024], f32, tag="hp")
                for dc in range(8):
                    kt = kpool.tile([128, K], kdt, tag="keys")
                    nc.sync.dma_start(kt[:], keys_d[b, dc * 128 : (dc + 1) * 128, :])
                    w_sl = w1t_sb[:, dc * H : (dc + 1) * H]
                    nc.tensor.matmul(
                        hp[:, 0:512], lhsT=w_sl, rhs=kt[:, 0:512],
                        start=(dc == 0), stop=(dc == 7),
                    )
                    nc.tensor.matmul(
                        hp[:, 512:1024], lhsT=w_sl, rhs=kt[:, 512:1024],
                        start=(dc == 0), stop=(dc == 7),
                    )
                # relu(h + b1): PSUM -> SBUF on DVE (cast to kdt for matmul-2)
                h0 = hpool.tile([H, 512], kdt, tag="h")
                h1 = hpool.tile([H, 512], kdt, tag="h")
                nc.vector.tensor_scalar(
                    h0[:], hp[:, 0:512], b1_sb[:], 0.0,
                    op0=mybir.AluOpType.add, op1=mybir.AluOpType.max,
                )
                nc.vector.tensor_scalar(
                    h1[:], hp[:, 512:1024], b1_sb[:], 0.0,
                    op0=mybir.AluOpType.add, op1=mybir.AluOpType.max,
                )
                # s_row over the full row (shares PSUM slots with hp)
                sr_ps = hp_pool.tile([1, 1024], f32, tag="hp")
                nc.tensor.matmul(
                    sr_ps[0:1, 0:512], lhsT=w2_sb[:], rhs=h0[:],
                    start=True, stop=True,
                )
                nc.tensor.matmul(
                    sr_ps[0:1, 512:1024], lhsT=w2_sb[:], rhs=h1[:],
                    start=True, stop=True,
                )
                # padded score rows -> one fused tile: [-(s_pos+pad), s_neg+pad]
                ppr_sb = spool.tile([1, Jmax], f32, tag="ppr")
                nc.gpsimd.dma_start(ppr_sb[:], ppr_d[b : b + 1, :])
                ngr_sb = spool.tile([1, nw], f32, tag="ngr")
                nc.gpsimd.dma_start(ngr_sb[:], negrow_d[b : b + 1, :])
                exin_sb = spool.tile([1, Jmax + nw], f32, tag="exin")
                nc.vector.scalar_tensor_tensor(
                    exin_sb[0:1, 0:Jmax], sr_ps[0:1, 0:Jmax], -1.0, ppr_sb[:],
                    op0=mybir.AluOpType.mult, op1=mybir.AluOpType.subtract,
                )
                nc.vector.tensor_add(
                    exin_sb[0:1, Jmax : Jmax + nw], sr_ps[0:1, nst:K], ngr_sb[:]
                )
                # e^{-(s_pos+pad)} | e^{s_neg+pad} in one bf16 row
                eall_sb = spool.tile([1, Jmax + nw], kdt, tag="eall")
                nc.scalar.activation(
                    eall_sb[:], exin_sb[:],
                    mybir.ActivationFunctionType.Exp, scale=1.0,
                )
                # outer products e^{-s_p} x e^{s_n} on PE (K=1 matmuls into
                # PSUM), then per-chunk Ln(x+1) on ACT reading PSUM directly
                for c in range(npch):
                    tp_ps = tp_pool.tile([128, nw], f32, tag="tp")
                    lw = eall_sb[0:1, c * 128 : (c + 1) * 128]
                    for s0 in range(0, nw, 512):
                        s1 = min(s0 + 512, nw)
                        nc.tensor.matmul(
                            tp_ps[:, s0:s1], lhsT=lw,
                            rhs=eall_sb[0:1, Jmax + s0 : Jmax + s1],
                            start=True, stop=True,
                        )
                    lout_sb = spool.tile([128, nw], f32, tag="lout")
                    nc.scalar.activation(
                        lout_sb[:], tp_ps[:],
                        mybir.ActivationFunctionType.Ln,
                        bias=1.0, scale=1.0,
                        accum_out=acc_sb[:, b * npch + c : b * npch + c + 1],
                    )

            nc.sync.dma_start(out_d[:], acc_sb[:])

    nc.compile()
    return nc


def kernel(keys, tgts, knn_tgts, mask, W1, b1, W2, b2, _profile=False):
    import ml_dtypes

    from concourse.bass_utils import run_bass_kernel_spmd

    keys = np.asarray(keys, dtype=np.float32)
    tgts = np.asarray(tgts)
    knn_tgts = np.asarray(knn_tgts)
    mask = np.asarray(mask).astype(bool)
    W1 = np.asarray(W1, dtype=np.float32)
    b1 = np.asarray(b1, dtype=np.float32)
    W2 = np.asarray(W2, dtype=np.float32)

    # ---- host-side label/permutation prep ----
    y = knn_tgts == tgts[:, None]
    pos = y & mask
    neg = (~y) & mask
    P = pos.sum(axis=1)
    N_ = neg.sum(axis=1)
    cnt = float((P.astype(np.int64) * N_.astype(np.int64)).sum())

    # stable order: positives, negatives, masked-out
    rank = np.where(pos, 0, np.where(neg, 1, 2)).astype(np.int8)
    order = np.argsort(rank, axis=1, kind="stable")  # [B, K]

    Pmax = int(P.max())
    Pmin = int(P.min())
    assert Pmax <= 512, f"positive count {Pmax} > 512 unsupported"
    Jmax = min(512, ((Pmax + 127) // 128) * 128)
    npch = Jmax // 128
    nst = min(Pmin, 512)  # negative free region start (s_row slice origin)
    nw = K - nst

    # permuted, transposed keys: [B, D, K]
    keys_perm = np.take_along_axis(keys, order[:, :, None], axis=1)  # [B, K, D]
    keys_t = np.ascontiguousarray(keys_perm.transpose(0, 2, 1))
    kdt = ml_dtypes.bfloat16 if USE_BF16 else np.float32
    keys_t = keys_t.astype(kdt)

    # pads in permuted coordinates
    kidx = np.arange(K)[None, :]
    pospad = np.where(kidx < P[:, None], 0.0, PAD).astype(np.float32)  # [B, K]
    negpad = np.where(
        (kidx >= P[:, None]) & (kidx < (P + N_)[:, None]), 0.0, -PAD
    ).astype(np.float32)
    ppr = np.ascontiguousarray(pospad[:, :Jmax])  # [B, Jmax]
    negrow = np.ascontiguousarray(negpad[:, nst:])  # [B, nw]

    w1t = np.ascontiguousarray(W1.T).astype(kdt)  # [D, H]
    w2c = np.ascontiguousarray(W2.reshape(1, H).T).astype(kdt)  # [H, 1]
    b1c = np.ascontiguousarray(b1.reshape(H, 1))

    key = (Jmax, nst, USE_BF16)
    if key not in _cache:
        _cache[key] = _build_program(Jmax, nst, USE_BF16)
    nc = _cache[key]

    in_maps = []
    for c in range(N_CORES):
        sl = slice(c * BPC, (c + 1) * BPC)
        in_maps.append(
            {
                "keys_t": keys_t[sl],
                "w1t": w1t,
                "w2c": w2c,
                "b1c": b1c,
                "ppr": ppr[sl],
                "negrow": negrow[sl],
            }
        )

    res = run_bass_kernel_spmd(
        nc, in_maps, list(range(N_CORES)), trace=bool(_profile)
    )
    total = 0.0
    for r in res.results:
        total += float(r["acc_out"].astype(np.float64).sum())
    if _profile:
        print(f"HW exec time: {res.exec_time_ns} ns")
        globals()["_last_results"] = res
    loss = np.float64(total) / np.float64(cnt)
    return np.array(loss, dtype=np.float32)


# revision 32
# speedup vs baseline: 1.4495x; 1.0816x over previous
"""Trainium2 Bass kernel for the kNN pairwise-ranking loss.

Math: with y = (knn_tgts == tgts), the masked pairwise BCE-with-logits loss
over differing-label pairs (j > i) collapses to

    loss = sum_b sum_{n in neg_b} sum_{p in pos_b} softplus(s_n - s_p) / cnt
    cnt  = sum_b |pos_b| * |neg_b|

because for a (pos, neg) pair the per-pair term is softplus(s_neg - s_pos)
regardless of orientation, and b2 cancels in score differences.

Host side: per batch row, permute keys so positives come first, then
negatives, then masked-out entries.  Additive pad vectors (+PAD on
non-positives, -PAD on non-negatives) push padded scores far out so their
softplus contribution underflows to exactly ln(1) = 0; the device then just
sums a dense [pos-chunks x neg-width] softplus block with no masking.

Device (SPMD over 8 cores, 4 batch rows each):
  phase A (per row): h = relu(W1 @ keys^T + b1) via PE (bf16 in, f32 psum),
                     s_row [1,K] via two small PE matmuls off the bf16 h.
  phase B (per row): softplus(s_n - s_p) = ln(1 + e^{s_n} * e^{-s_p}).
                     One fused ACT exp produces both e^{-(s_pos+pad)} [1,Jmax]
                     and e^{s_neg+pad} [1,nw]; GPSIMD partition-broadcasts the
                     row factor, a DRAM round-trip reshapes the column factor
                     to [128,npch]; DVE per-partition multiplies form the
                     outer products; one Ln(x+1) pass per row with accum_out
                     yields the per-partition sums.  Exp and Ln are forced
                     into one ACT table set (see _patch_act_tables) so only a
                     single ACT_TABLE_LOAD is issued.
Host gathers [128, BPC] partial sums, reduces, divides by cnt.
"""

import numpy as np

B, K, D, H = 32, 1024, 1024, 100
N_CORES = 8
BPC = B // N_CORES  # batch rows per core
PAD = 60.0
USE_BF16 = True
USE_FP8 = True

_cache = {}
_act_patched = False


def _patch_act_tables():
    """Make Exp/Ln resolve to the single combined ACT table set."""
    global _act_patched
    if _act_patched:
        return
    import concourse.bacc as bacc
    import concourse.hw_specs as hw_specs
    import concourse.mybir as mybir

    orig = hw_specs.get_activation_tables
    combined = "natural_log_exp_and_others"

    def patched(arch):
        tabs = orig(arch)
        out = {}
        for name, funcs in tabs.items():
            f = set(funcs)
            if name != combined and combined in tabs:
                f.discard(mybir.ActivationFunctionType.Exp)
                f.discard(mybir.ActivationFunctionType.Ln)
            out[name] = f
        return out

    hw_specs.get_activation_tables = patched
    bacc.get_activation_tables = patched
    _act_patched = True


def _build_program(Jmax, nst, use_bf16, use_fp8):
    import concourse.bacc as bacc
    import concourse.mybir as mybir
    import concourse.tile as tile

    _patch_act_tables()

    f32 = mybir.dt.float32
    kdt = mybir.dt.bfloat16 if use_bf16 else f32
    edt = mybir.dt.float8e4 if use_fp8 else kdt
    npch = Jmax // 128  # positive-side partition chunks
    nw = K - nst  # negative-side free width
    ndc = 4 if use_fp8 else 8  # contraction chunks (256 wide with DoubleRow)

    nc = bacc.Bacc(
        "TRN2",
        target_bir_lowering=False,
        debug=False,
        enable_asserts=False,
        num_devices=N_CORES,
    )

    keys_d = nc.dram_tensor("keys_t", [BPC, D, K], edt, kind="ExternalInput").ap()
    hpad = 112  # padded per-subrow weight stride (DoubleRow needs step%16==0)
    wchunk = 2 * hpad if use_fp8 else H
    w1t_d = nc.dram_tensor("w1t", [ndc, 128, wchunk], edt, kind="ExternalInput").ap()
    w2_d = nc.dram_tensor("w2c", [H, 1], kdt, kind="ExternalInput").ap()
    b1_d = nc.dram_tensor("b1c", [H, 1], f32, kind="ExternalInput").ap()
    ppr_d = nc.dram_tensor("ppr", [BPC, Jmax], f32, kind="ExternalInput").ap()
    negrow_d = nc.dram_tensor("negrow", [BPC, nw], f32, kind="ExternalInput").ap()
    out_d = nc.dram_tensor(
        "acc_out", [128, BPC * npch], f32, kind="ExternalOutput"
    ).ap()

    with tile.TileContext(nc) as tc:
        with (
            tc.tile_pool(name="const", bufs=1) as cpool,
            tc.tile_pool(name="keys", bufs=28) as kpool,
            tc.tile_pool(name="h", bufs=4) as hpool,
            tc.tile_pool(name="svec", bufs=3) as spool,
            tc.tile_pool(name="hp", bufs=2, space="PSUM") as hp_pool,
            tc.tile_pool(name="tp", bufs=2, space="PSUM") as tp_pool,
        ):
            # ---- constants (scalar+gpsimd queues: sync queue is for keys) ----
            w1t_sb = cpool.tile([128, ndc * wchunk], edt, tag="w1t")
            for dc in range(ndc):
                eng = nc.scalar if dc % 2 == 0 else nc.gpsimd
                eng.dma_start(
                    w1t_sb[:, dc * wchunk : (dc + 1) * wchunk],
                    w1t_d[dc, :, :],
                )
            w2_sb = cpool.tile([H, 1], kdt, tag="w2")
            nc.scalar.dma_start(w2_sb[:], w2_d[:])
            b1_sb = cpool.tile([H, 1], f32, tag="b1")
            nc.scalar.dma_start(b1_sb[:], b1_d[:])
            acc_sb = cpool.tile([128, BPC * npch], f32, tag="acc")

            for b in range(BPC):
                # ---- phase A: MLP scores ----
                hp = hp_pool.tile([H, 1024], f32, tag="hp")
                dstep = D // ndc
                for dc in range(ndc):
                    if use_fp8:
                        kt = kpool.tile([128, 2 * K], edt, tag="keys")
                        nc.sync.dma_start(
                            kt[:].rearrange("p (i k) -> p i k", i=2),
                            keys_d[b, dc * 256 : (dc + 1) * 256, :].rearrange(
                                "(i p) k -> p i k", i=2
                            ),
                        )
                        w_sl = w1t_sb[
                            :, dc * wchunk : (dc + 1) * wchunk
                        ].rearrange("p (i m) -> p i m", i=2)[:, :, 0:H]
                        kt3 = kt[:].rearrange("p (i k) -> p i k", i=2)
                        for kh in range(2):
                            nc.tensor.matmul(
                                hp[:, kh * 512 : (kh + 1) * 512],
                                lhsT=w_sl,
                                rhs=kt3[:, :, kh * 512 : (kh + 1) * 512],
                                start=(dc == 0), stop=(dc == ndc - 1),
                                perf_mode=mybir.MatmulPerfMode.DoubleRow,
                            )
                    else:
                        kt = kpool.tile([128, K], kdt, tag="keys")
                        nc.sync.dma_start(
                            kt[:], keys_d[b, dc * 128 : (dc + 1) * 128, :]
                        )
                        w_sl = w1t_sb[:, dc * H : (dc + 1) * H]
                        nc.tensor.matmul(
                            hp[:, 0:512], lhsT=w_sl, rhs=kt[:, 0:512],
                            start=(dc == 0), stop=(dc == ndc - 1),
                        )
                        nc.tensor.matmul(
                            hp[:, 512:1024], lhsT=w_sl, rhs=kt[:, 512:1024],
                            start=(dc == 0), stop=(dc == ndc - 1),
                        )
                # relu(h + b1): PSUM -> SBUF on DVE (cast to kdt for matmul-2)
                h0 = hpool.tile([H, 512], kdt, tag="h")
                h1 = hpool.tile([H, 512], kdt, tag="h")
                nc.vector.tensor_scalar(
                    h0[:], hp[:, 0:512], b1_sb[:], 0.0,
                    op0=mybir.AluOpType.add, op1=mybir.AluOpType.max,
                )
                nc.vector.tensor_scalar(
                    h1[:], hp[:, 512:1024], b1_sb[:], 0.0,
                    op0=mybir.AluOpType.add, op1=mybir.AluOpType.max,
                )
                # s_row over the full row (shares PSUM slots with hp)
                sr_ps = hp_pool.tile([1, 1024], f32, tag="hp")
                nc.tensor.matmul(
                    sr_ps[0:1, 0:512], lhsT=w2_sb[:], rhs=h0[:],
                    start=True, stop=True,
                )
                nc.tensor.matmul(
                    sr_ps[0:1, 512:1024], lhsT=w2_sb[:], rhs=h1[:],
                    start=True, stop=True,
                )
                # padded score rows -> one fused tile: [-(s_pos+pad), s_neg+pad]
                ppr_sb = spool.tile([1, Jmax], f32, tag="ppr")
                nc.gpsimd.dma_start(ppr_sb[:], ppr_d[b : b + 1, :])
                ngr_sb = spool.tile([1, nw], f32, tag="ngr")
                nc.gpsimd.dma_start(ngr_sb[:], negrow_d[b : b + 1, :])
                exin_sb = spool.tile([1, Jmax + nw], f32, tag="exin")
                nc.vector.scalar_tensor_tensor(
                    exin_sb[0:1, 0:Jmax], sr_ps[0:1, 0:Jmax], -1.0, ppr_sb[:],
                    op0=mybir.AluOpType.mult, op1=mybir.AluOpType.subtract,
                )
                nc.vector.tensor_add(
                    exin_sb[0:1, Jmax : Jmax + nw], sr_ps[0:1, nst:K], ngr_sb[:]
                )
                # e^{-(s_pos+pad)} | e^{s_neg+pad} in one bf16 row
                eall_sb = spool.tile([1, Jmax + nw], kdt, tag="eall")
                nc.scalar.activation(
                    eall_sb[:], exin_sb[:],
                    mybir.ActivationFunctionType.Exp, scale=1.0,
                )
                # outer products e^{-s_p} x e^{s_n} on PE (K=1 matmuls into
                # PSUM), then per-chunk Ln(x+1) on ACT reading PSUM directly
                for c in range(npch):
                    tp_ps = tp_pool.tile([128, nw], f32, tag="tp")
                    lw = eall_sb[0:1, c * 128 : (c + 1) * 128]
                    for s0 in range(0, nw, 512):
                        s1 = min(s0 + 512, nw)
                        nc.tensor.matmul(
                            tp_ps[:, s0:s1], lhsT=lw,
                            rhs=eall_sb[0:1, Jmax + s0 : Jmax + s1],
                            start=True, stop=True,
                        )
                    lout_sb = spool.tile([128, nw], f32, tag="lout")
                    nc.scalar.activation(
                        lout_sb[:], tp_ps[:],
                        mybir.ActivationFunctionType.Ln,
                        bias=1.0, scale=1.0,
                        accum_out=acc_sb[:, b * npch + c : b * npch + c + 1],
                    )

            nc.sync.dma_start(out_d[:], acc_sb[:])

    nc.compile()
    return nc


def kernel(keys, tgts, knn_tgts, mask, W1, b1, W2, b2, _profile=False):
    import ml_dtypes

    from concourse.bass_utils import run_bass_kernel_spmd

    keys = np.asarray(keys, dtype=np.float32)
    tgts = np.asarray(tgts)
    knn_tgts = np.asarray(knn_tgts)
    mask = np.asarray(mask).astype(bool)
    W1 = np.asarray(W1, dtype=np.float32)
    b1 = np.asarray(b1, dtype=np.float32)
    W2 = np.asarray(W2, dtype=np.float32)

    # ---- host-side label/permutation prep ----
    y = knn_tgts == tgts[:, None]
    pos = y & mask
    neg = (~y) & mask
    P = pos.sum(axis=1)
    N_ = neg.sum(axis=1)
    cnt = float((P.astype(np.int64) * N_.astype(np.int64)).sum())

    # stable order: positives, negatives, masked-out
    rank = np.where(pos, 0, np.where(neg, 1, 2)).astype(np.int8)
    order = np.argsort(rank, axis=1, kind="stable")  # [B, K]

    Pmax = int(P.max())
    Pmin = int(P.min())
    assert Pmax <= 512, f"positive count {Pmax} > 512 unsupported"
    Jmax = min(512, ((Pmax + 127) // 128) * 128)
    npch = Jmax // 128
    nst = min(Pmin, 512)  # negative free region start (s_row slice origin)
    nw = K - nst

    # permuted, transposed keys: [B, D, K]
    keys_perm = np.take_along_axis(keys, order[:, :, None], axis=1)  # [B, K, D]
    keys_t = np.ascontiguousarray(keys_perm.transpose(0, 2, 1))
    kdt = ml_dtypes.bfloat16 if USE_BF16 else np.float32
    edt = ml_dtypes.float8_e4m3 if USE_FP8 else kdt
    keys_t = keys_t.astype(edt)

    # pads in permuted coordinates
    kidx = np.arange(K)[None, :]
    pospad = np.where(kidx < P[:, None], 0.0, PAD).astype(np.float32)  # [B, K]
    negpad = np.where(
        (kidx >= P[:, None]) & (kidx < (P + N_)[:, None]), 0.0, -PAD
    ).astype(np.float32)
    ppr = np.ascontiguousarray(pospad[:, :Jmax])  # [B, Jmax]
    negrow = np.ascontiguousarray(negpad[:, nst:])  # [B, nw]

    if USE_FP8:
        # scale W1 by 16 into fp8's sweet spot; fold 1/16 into W2 and 16 into
        # b1 (exact through relu's positive homogeneity)
        ndc = 4
        hpad = 112
        w1s = (W1.T * 16.0).astype(np.float32)  # [D, H]
        # [ndc, 128, 2, hpad]: d = dc*256 + i*128 + p, H cols + zero pad
        w4 = np.zeros((ndc, 2, 128, hpad), dtype=np.float32)
        w4[:, :, :, :H] = w1s.reshape(ndc, 2, 128, H)
        w1t = np.ascontiguousarray(
            w4.transpose(0, 2, 1, 3).reshape(ndc, 128, 2 * hpad)
        ).astype(edt)
        w2c = np.ascontiguousarray(W2.reshape(1, H).T / 16.0).astype(kdt)  # [H, 1]
        b1c = np.ascontiguousarray(b1.reshape(H, 1) * 16.0)
    else:
        w1t = np.ascontiguousarray(W1.T).astype(kdt)  # [D, H]
        w2c = np.ascontiguousarray(W2.reshape(1, H).T).astype(kdt)  # [H, 1]
        b1c = np.ascontiguousarray(b1.reshape(H, 1))

    key = (Jmax, nst, USE_BF16, USE_FP8)
    if key not in _cache:
        _cache[key] = _build_program(Jmax, nst, USE_BF16, USE_FP8)
    nc = _cache[key]

    in_maps = []
    for c in range(N_CORES):
        sl = slice(c * BPC, (c + 1) * BPC)
        in_maps.append(
            {
                "keys_t": keys_t[sl],
                "w1t": w1t,
                "w2c": w2c,
                "b1c": b1c,
                "ppr": ppr[sl],
                "negrow": negrow[sl],
            }
        )

    res = run_bass_kernel_spmd(
        nc, in_maps, list(range(N_CORES)), trace=bool(_profile)
    )
    total = 0.0
    for r in res.results:
        total += float(r["acc_out"].astype(np.float64).sum())
    if _profile:
        print(f"HW exec time: {res.exec_time_ns} ns")
        globals()["_last_results"] = res
    loss = np.float64(total) / np.float64(cnt)
    return np.array(loss, dtype=np.float32)
